# revision 1
# baseline (speedup 1.0000x reference)
"""DetConB loss (nn_DetConBLoss) on 8 TRN2 NeuronCores via Bass/Tile.

Strategy (data-parallel over batch, targets replicated):
  - Host: l2-normalize preds/targets in f32, flatten to (4096, 256),
    transpose to (d, rows), cast bf16. Core c owns pred rows
    [c*512, (c+1)*512). Each core receives the full targets with columns
    rolled by c*512 so its own-image diagonal band sits at a fixed,
    compile-time-constant column range (the program is SPMD-identical).
  - Device (per core): for each of the 4 pred x target combinations,
    a (512 x 4096) fp8 DoubleRow matmul (K=256 in one pass, fp32 PSUM
    accum) fused with exp(scale*x) on ScalarE at its roofline; row-sums
    via ACTIVATE's accumulator on one PSUM buffer and a DVE reduce on
    the other. Only the 32 KB of row-sum partials leave the device.
  - Host: the 16x16 own-image diagonal dot blocks (recomputed from the
    same fp8 inputs, ~0.4% of total FLOPs), masks from the roi indices,
    positive-pair sums, the -inf masking correction (subtract the exp of
    masked entries from the denominators), log, and the final mean.

All 34.4 GFLOP of matmul and the 67M-element exp run on device; the host
handles O(b*n^2)-scale arithmetic.
"""
import numpy as np
import ml_dtypes

import concourse.bacc as bacc
import concourse.mybir as mybir
import concourse.tile as tile
from concourse.bass_utils import run_bass_kernel_spmd

TEMP = 0.1
EPS = 1e-11
SCALE = float(np.float32(1.0 / (TEMP + EPS)))
NCORES = 8
B, N, D = 256, 16, 256
R = B * N          # 4096 flat rows
RPC = R // NCORES  # 512 rows per core
MT = RPC // 128    # 4 row-tiles of 128 per core
BF16 = mybir.dt.bfloat16
FP8 = mybir.dt.float8e4
NPFP8 = ml_dtypes.float8_e4m3
F32 = mybir.dt.float32
I32 = mybir.dt.int32
# Schraudolph fast-exp: exp(s*x) ~= bitcast_f32(int32(x*SA + SB))
SA = float(np.float32((2**23 / np.log(2.0)) * (1.0 / (0.1 + 1e-11))))
SB = float(np.float32(127 * 2**23 - 486411))


def build_nc():
    """Build + schedule + compile the SPMD per-core Bass program."""
    nc = bacc.Bacc("TRN2", target_bir_lowering=False, debug=False,
                   num_devices=NCORES)

    p_dram = [nc.dram_tensor(f"p{i + 1}t", [D, RPC], FP8, kind="ExternalInput")
              for i in range(2)]
    t_dram = [nc.dram_tensor(f"t{i + 1}t", [D, R], FP8, kind="ExternalInput")
              for i in range(2)]
    sacc = nc.dram_tensor("sacc", [128, 80], F32, kind="ExternalOutput")

    with tile.TileContext(nc) as tc:
        with (
            tc.tile_pool(name="const", bufs=1) as const_pool,
            tc.tile_pool(name="psum", bufs=2, space="PSUM") as psum_pool,
            tc.tile_pool(name="scratch", bufs=6) as scratch_pool,
        ):
            # Persistent SBUF: targets as [K=128 partitions, kchunk*R + col],
            # preds as [128, kchunk*RPC + col].
            t_sb = [const_pool.tile([128, 2 * R], FP8, name=f"t_sb{i}", tag=f"t{i}")
                    for i in range(2)]
            p_sb = [const_pool.tile([128, 2 * RPC], FP8, name=f"p_sb{i}", tag=f"p{i}")
                    for i in range(2)]

            # All 32 row-sum partials live in one persistent strip; a single
            # 32 KB DMA ships them at the end (col 2*it = g0 via DVE reduce,
            # col 2*it+1 = g1 via ACT accumulator).
            strip = const_pool.tile([128, 80], F32, name="strip", tag="strip")
            nc.vector.memset(strip, 0.0)
            # Explicit zero-bias AP: a float bias would be lowered through the
            # const-AP machinery, whose TENSOR_LOAD sits in the preamble.
            zbias = const_pool.tile([128, 1], F32, name="zbias", tag="zbias")
            nc.vector.memset(zbias, 0.0)
            # Warm the exp table set during the input-DMA window so the first
            # real ACTIVATE does not pay the ~2.7us ACT_TABLE_LOAD.
            nc.scalar.activation(strip[:, 0:2], strip[:, 0:2],
                                 mybir.ActivationFunctionType.Exp, bias=zbias)
            nc.vector.memset(strip[:, 0:2], 0.0)

            # Input DMAs on the sync (HWDGE) queue, ordered by first use:
            # p1 + the first two t1 chunks gate iteration 0.
            def load_t(tsel, k, g):
                cs = g * 2048
                nc.sync.dma_start(
                    out=t_sb[tsel][:, k * R + cs: k * R + cs + 2048],
                    in_=t_dram[tsel][k * 128:(k + 1) * 128, cs:cs + 2048])

            def load_p(px):
                nc.sync.dma_start(
                    out=p_sb[px].rearrange("p (k c) -> p k c", k=2),
                    in_=p_dram[px].ap().rearrange("(k p) c -> p k c", p=128))

            def load_t_fine(tsel, k, g, q):
                cs = g * 2048 + q * 1024
                nc.sync.dma_start(
                    out=t_sb[tsel][:, k * R + cs: k * R + cs + 1024],
                    in_=t_dram[tsel][k * 128:(k + 1) * 128, cs:cs + 1024])

            load_p(0)
            for q in range(2):
                load_t_fine(0, 0, 0, q)
                load_t_fine(0, 1, 0, q)
            load_t(0, 0, 1)
            load_t(0, 1, 1)
            load_p(1)
            for k in range(2):
                for g in range(2):
                    load_t(1, k, g)

            # tsel outer: the first 8 halves consume only t1, so the t2
            # load (2 MB) hides behind ~28 us of compute.
            for tsel in range(2):
                for px in range(2):
                    for mt in range(MT):
                        it = tsel * 8 + px * MT + mt
                        # One 4096-col half = both PSUM buffers. k-outer so 8
                        # consecutive matmuls share the stationary weights and
                        # stream back-to-back (no LDWEIGHTS-induced drain).
                        ps = [psum_pool.tile([128, 2048], F32, name=f"ps{h}",
                                             tag="ps")
                              for h in range(2)]
                        # fp8 DoubleRow: both 128-deep K chunks contract in a
                        # single pass (lhsT/rhs carry the k pair on a middle
                        # AP dim), so each 512-col tile is one matmul.
                        lhs3 = p_sb[px].rearrange("p (k c) -> p k c", k=2)
                        rhs3 = t_sb[tsel].rearrange("p (k c) -> p k c", k=2)
                        for g in range(2):
                            for j in range(4):
                                c0 = g * 2048 + j * 512
                                nc.tensor.matmul(
                                    ps[g][:, j * 512:(j + 1) * 512],
                                    lhs3[:, :, mt * 128:(mt + 1) * 128],
                                    rhs3[:, :, c0:c0 + 512],
                                    start=True, stop=True,
                                    perf_mode=mybir.MatmulPerfMode.DoubleRow)
                        # (The own-image diagonal blocks are recomputed on
                        # the host from the same fp8 inputs — no band output.)
                        # g0: ACT accumulator (its READ_ACCUMULATOR lands
                        # mid-period, off the inter-iteration critical path);
                        # g1: exp on ACT, row-sum on the otherwise-idle DVE.
                        scr0 = scratch_pool.tile([128, 2048], BF16, name="scr0",
                                                 tag="scr")
                        nc.scalar.activation(
                            scr0, ps[0], mybir.ActivationFunctionType.Exp,
                            bias=zbias, scale=SCALE,
                            accum_out=strip[:, 2 * it:2 * it + 1])
                        scr1 = scratch_pool.tile([128, 2048], BF16, name="scr1",
                                                 tag="scr")
                        if it == 15:
                            # Final iteration: DVE work would sit on the
                            # kernel-exit path; the ACT accumulator's read-out
                            # is cheaper there.
                            nc.scalar.activation(
                                scr1, ps[1], mybir.ActivationFunctionType.Exp,
                                bias=zbias, scale=SCALE,
                                accum_out=strip[:, 2 * it + 1:2 * it + 2])
                        else:
                            # Half of g1 goes through a Schraudolph fast-exp
                            # on the now-idle DVE (int-converting multiply-add
                            # + reduce of the bitcast), shortening the
                            # critical ScalarE chain to 2048+1024 columns.
                            sch = scratch_pool.tile([128, 1024], I32,
                                                    name="sch", tag="sch")
                            nc.vector.tensor_scalar(
                                sch, ps[1][:, 1024:2048], SA, SB,
                                op0=mybir.AluOpType.mult,
                                op1=mybir.AluOpType.add)
                            nc.vector.tensor_reduce(
                                strip[:, 64 + it:65 + it], sch.bitcast(F32),
                                axis=mybir.AxisListType.X, op=mybir.AluOpType.add)
                            nc.scalar.activation(
                                scr1[:, 0:1024], ps[1][:, 0:1024],
                                mybir.ActivationFunctionType.Exp,
                                bias=zbias, scale=SCALE)
                            nc.vector.tensor_reduce(
                                strip[:, 2 * it + 1:2 * it + 2], scr1[:, 0:1024],
                                axis=mybir.AxisListType.X, op=mybir.AluOpType.add)
            # Final strip DMA on the sync HWDGE queue: the gpsimd SWDGE
            # drain at kernel exit is ~2.4us when it must wait for this
            # transfer; HWDGE drains in ~0.1us.
            nc.sync.dma_start(out=sacc.ap(), in_=strip)

    nc.compile()
    return nc


_NC = None


def _get_nc():
    global _NC
    if _NC is None:
        _NC = build_nc()
    return _NC


def _l2norm(x):
    return x / np.linalg.norm(x, axis=-1, keepdims=True)


def host_prep(pred1, pred2, target1, target2):
    p1t = _l2norm(np.asarray(pred1, np.float32)).reshape(R, D).T.astype(NPFP8)
    p2t = _l2norm(np.asarray(pred2, np.float32)).reshape(R, D).T.astype(NPFP8)
    t1t = _l2norm(np.asarray(target1, np.float32)).reshape(R, D).T.astype(NPFP8)
    t2t = _l2norm(np.asarray(target2, np.float32)).reshape(R, D).T.astype(NPFP8)
    # Raw own-image diagonal dot blocks (b, n, m), fp8-quantized operands in
    # f32 — the same products the device computes, ~0.4% of total FLOPs.
    pf = [p1t.T.astype(np.float32).reshape(B, N, D),
          p2t.T.astype(np.float32).reshape(B, N, D)]
    tf = [t1t.T.astype(np.float32).reshape(B, N, D),
          t2t.T.astype(np.float32).reshape(B, N, D)]
    diag = [[np.einsum('bnd,bmd->bnm', pf[px], tf[ts]).astype(np.float32)
             for ts in range(2)] for px in range(2)]
    in_maps = []
    for c in range(NCORES):
        r0 = c * RPC
        in_maps.append({
            "p1t": np.ascontiguousarray(p1t[:, r0:r0 + RPC]),
            "p2t": np.ascontiguousarray(p2t[:, r0:r0 + RPC]),
            "t1t": np.ascontiguousarray(np.concatenate([t1t[:, r0:], t1t[:, :r0]], axis=1)),
            "t2t": np.ascontiguousarray(np.concatenate([t2t[:, r0:], t2t[:, :r0]], axis=1)),
        })
    return in_maps, diag


def host_post(results, diag, pind1, pind2, tind1, tind2):
    S = np.zeros((2, R), np.float64)
    for c, res in enumerate(results):
        sacc = np.asarray(res["sacc"])
        for px in range(2):
            for mt in range(MT):
                r0 = c * RPC + mt * 128
                cols = [2 * (tsel * 8 + px * MT + mt) + g
                        for tsel in range(2) for g in range(2)]
                cols += [64 + tsel * 8 + px * MT + mt for tsel in range(2)]
                S[px, r0:r0 + 128] = sacc[:, cols].astype(np.float64).sum(axis=1)
    sc = np.float32(SCALE)
    D_aa = sc * diag[0][0]
    D_ab = sc * diag[0][1]
    D_ba = sc * diag[1][0]
    D_bb = sc * diag[1][1]

    f32 = np.float32
    pind1, pind2 = np.asarray(pind1), np.asarray(pind2)
    tind1, tind2 = np.asarray(tind1), np.asarray(tind2)
    same_aa = (pind1[:, :, None] == tind1[:, None, :]).astype(f32)
    same_ab = (pind1[:, :, None] == tind2[:, None, :]).astype(f32)
    same_ba = (pind2[:, :, None] == tind1[:, None, :]).astype(f32)
    same_bb = (pind2[:, :, None] == tind2[:, None, :]).astype(f32)

    S0 = S[0].reshape(B, N)
    S1 = S[1].reshape(B, N)
    corr0 = (same_aa * np.exp(D_aa.astype(np.float64))).sum(-1)
    corr1 = (same_bb * np.exp(D_bb.astype(np.float64))).sum(-1)
    lse0 = np.log(S0 - corr0)
    lse1 = np.log(S1 - corr1)

    num_pos0 = same_ab.sum(-1)
    num_pos1 = same_ba.sum(-1)
    pos_sum0 = (same_ab * D_ab).sum(-1)
    pos_sum1 = (same_ba * D_ba).sum(-1)

    area0 = (pind1[:, :, None] == pind1[:, None, :]).astype(f32).sum(-1)
    area1 = (pind2[:, :, None] == pind2[:, None, :]).astype(f32).sum(-1)
    w0 = (num_pos0 > 0.001).astype(f32) / area0
    w1 = (num_pos1 > 0.001).astype(f32) / area1

    ce0 = -w0 * (pos_sum0 - num_pos0 * lse0) / np.maximum(num_pos0, 1.0)
    ce1 = -w1 * (pos_sum1 - num_pos1 * lse1) / np.maximum(num_pos1, 1.0)
    return np.float32(ce0.mean() + ce1.mean())


def run_hw(inputs, trace=False):
    nc = _get_nc()
    in_maps, diag = host_prep(inputs["pred1"], inputs["pred2"],
                              inputs["target1"], inputs["target2"])
    last_err = None
    for attempt in range(3):
        try:
            res = run_bass_kernel_spmd(nc, in_maps,
                                       core_ids=list(range(NCORES)),
                                       trace=trace)
            break
        except Exception as e:  # transient NRT device errors recover on retry
            last_err = e
            import time
            time.sleep(20 * (attempt + 1))
    else:
        raise last_err
    loss = host_post(res.results, diag, inputs["pind1"], inputs["pind2"],
                     inputs["tind1"], inputs["tind2"])
    return loss, res


def kernel(**inputs):
    loss, _ = run_hw(inputs, trace=False)
    return loss



# revision 5
# speedup vs baseline: 1.1318x; 1.1318x over previous
"""DetConB loss (nn_DetConBLoss) on 8 TRN2 NeuronCores via Bass/Tile.

Strategy (data-parallel over batch, targets replicated):
  - Host: l2-normalize preds/targets in f32, flatten to (4096, 256),
    transpose to (d, rows), cast fp8e4m3. Core c owns pred rows
    [c*512, (c+1)*512). Each core receives the full targets with columns
    rolled by c*512 so its own-image diagonal band sits at a fixed,
    compile-time-constant column range (the program is SPMD-identical).
  - Device (per core): 32 units, each a (128 pred x 2048 target) slab:
    fp8 DoubleRow matmuls (K=256 in one pass, f32 PSUM) + one of two
    row-sum consumers, statically balanced to the engines' measured
    rates (ACT ~2.45us/slab incl READ_ACCUMULATOR, DVE ~5.4us/slab):
      * 22 ACT units: exp via ScalarE ACTIVATE with the free in-op
        accumulator (accum_out) - one pass, fused row-sum.
      * 10 DVE units: Schraudolph fast-exp on DVE (int-converting
        multiply-add to an i32 whose bits are the f32 exp) + bitcast
        tensor_reduce. Placed on target slabs that exclude both
        own-image diagonal bands, so the -inf correction on the host
        subtracts exact exps.
    Units are emitted in [A A A A D D] blocks: each engine's successive
    ops land on alternating PSUM slots, hiding every slot refill.
  - Host: the 16x16 own-image diagonal dot blocks (recomputed from the
    same fp8 inputs, ~0.4% of total FLOPs), masks from the roi indices,
    positive-pair sums, the -inf masking correction (subtract the exp of
    masked entries from the denominators), log, and the final mean.
"""
import numpy as np
import ml_dtypes

import concourse.bacc as bacc
import concourse.mybir as mybir
import concourse.tile as tile
from concourse.bass_utils import run_bass_kernel_spmd

TEMP = 0.1
EPS = 1e-11
SCALE = float(np.float32(1.0 / (TEMP + EPS)))
NCORES = 8
B, N, D = 256, 16, 256
R = B * N          # 4096 flat rows
RPC = R // NCORES  # 512 rows per core
BF16 = mybir.dt.bfloat16
FP8 = mybir.dt.float8e4
NPFP8 = ml_dtypes.float8_e4m3
F32 = mybir.dt.float32
I32 = mybir.dt.int32
# Schraudolph fast-exp: exp(s*x) ~= bitcast_f32(int32(x*SA + SB))
SA = float(np.float32((2**23 / np.log(2.0)) * (1.0 / (0.1 + 1e-11))))
SB = float(np.float32(127 * 2**23 - 486411))

# Unit table: unit (pt, h) = pred tile pt (px*4+mt) x target half-slab h
# (tsel = h//2, cols [2048*(h%2), +2048)).  DVE units sit on slabs with no
# own-image diagonal (h=1 for every pt, h=3 for pt 0 and 4): 22 ACT + 10 DVE.
D_UNITS = tuple([(pt, 1) for pt in range(8)] + [(0, 3), (4, 3)])
A_UNITS = tuple((pt, h) for pt in range(8) for h in (0, 2, 3)
                if (pt, h) not in D_UNITS)


def unit_sequence():
    """[A A A A D D] x 5 + [A A]: successive ops of each engine always land
    on alternating PSUM slots (position parity alternates per engine)."""
    a, d, seq = list(A_UNITS), list(D_UNITS), []
    while a or d:
        for _ in range(4):
            if a:
                seq.append((a.pop(0), "A"))
        for _ in range(2):
            if d:
                seq.append((d.pop(0), "D"))
    return seq


def build_nc():
    """Build + schedule + compile the SPMD per-core Bass program."""
    nc = bacc.Bacc("TRN2", target_bir_lowering=False, debug=False,
                   num_devices=NCORES)

    p_dram = nc.dram_tensor("pt", [D, 2 * RPC], FP8, kind="ExternalInput")
    t_dram = [nc.dram_tensor(f"t{i + 1}t", [D, R], FP8, kind="ExternalInput")
              for i in range(2)]
    sacc = nc.dram_tensor("sacc", [128, 32], F32, kind="ExternalOutput")

    with tile.TileContext(nc) as tc:
        with (
            tc.tile_pool(name="const", bufs=1) as const_pool,
            tc.tile_pool(name="psum", bufs=1, space="PSUM") as psum_pool,
            tc.tile_pool(name="sch", bufs=2) as sch_pool,
        ):
            # Persistent SBUF: targets as [K=128 partitions, kchunk*R + col],
            # preds as [128, kchunk*1024 + px*512 + col].
            t_sb = [const_pool.tile([128, 2 * R], FP8, name=f"t_sb{i}", tag=f"t{i}")
                    for i in range(2)]
            p_sb = const_pool.tile([128, 2 * 1024], FP8, name="p_sb", tag="p")
            t3 = [t_sb[i].rearrange("p (k c) -> p k c", k=2) for i in range(2)]
            p3 = p_sb.rearrange("p (k c) -> p k c", k=2)

            strip = const_pool.tile([128, 32], F32, name="strip", tag="strip")
            zbias = const_pool.tile([128, 1], F32, name="zbias", tag="zbias")
            scr = const_pool.tile([128, 2048], BF16, name="scr", tag="scr")
            nc.vector.memset(strip, 0.0)
            # Explicit zero-bias AP: a float bias would be lowered through the
            # const-AP machinery, whose TENSOR_LOAD sits in the preamble.
            nc.vector.memset(zbias, 0.0)
            # Warm the exp table set during the input-DMA window so the first
            # real ACTIVATE does not pay the ~2.7us ACT_TABLE_LOAD.
            nc.scalar.activation(strip[:, 0:2], strip[:, 0:2],
                                 mybir.ActivationFunctionType.Exp, bias=zbias)
            nc.vector.memset(strip[:, 0:2], 0.0)

            # PSUM: 2 ping-pong slots x [128, 2048] f32 = 8 banks exactly.
            ps = [psum_pool.tile([128, 2048], F32, name=f"ps{i}", tag=f"ps{i}")
                  for i in range(2)]

            # Input DMAs on the sync (HWDGE) queue, ordered by first use
            # (h-slab first-need order is 0, 2, 1, 3).
            def load_t(tsel, k, g):
                cs = g * 2048
                nc.sync.dma_start(
                    out=t_sb[tsel][:, k * R + cs: k * R + cs + 2048],
                    in_=t_dram[tsel][k * 128:(k + 1) * 128, cs:cs + 2048])

            nc.sync.dma_start(
                out=p3, in_=p_dram.ap().rearrange("(k p) c -> p k c", p=128))
            for g in range(2):
                for tsel in range(2):
                    for k in range(2):
                        load_t(tsel, k, g)

            def emit_mm(i, pt, h):
                tsel, g = h // 2, h % 2
                lhs = p3[:, :, pt * 128:(pt + 1) * 128]
                # fp8 DoubleRow: both 128-deep K chunks contract in a single
                # pass (lhsT/rhs carry the k pair on a middle AP dim).
                for j in range(4):
                    c0 = g * 2048 + j * 512
                    nc.tensor.matmul(
                        ps[i % 2][:, j * 512:(j + 1) * 512],
                        lhs, t3[tsel][:, :, c0:c0 + 512],
                        start=True, stop=True,
                        perf_mode=mybir.MatmulPerfMode.DoubleRow)

            seq = unit_sequence()
            ucol = {pth: u for u, (pth, _) in enumerate(seq)}
            for i, ((pt, h), kind) in enumerate(seq):
                emit_mm(i, pt, h)
                if kind == "A":
                    nc.scalar.activation(
                        scr, ps[i % 2], mybir.ActivationFunctionType.Exp,
                        bias=zbias, scale=SCALE,
                        accum_out=strip[:, i:i + 1])
                else:
                    sch = sch_pool.tile([128, 2048], I32, name="sch", tag="sch")
                    nc.vector.tensor_scalar(
                        sch, ps[i % 2], SA, SB,
                        op0=mybir.AluOpType.mult, op1=mybir.AluOpType.add)
                    nc.vector.tensor_reduce(
                        strip[:, i:i + 1], sch.bitcast(F32),
                        axis=mybir.AxisListType.X, op=mybir.AluOpType.add)
            # Final strip DMA on the sync HWDGE queue: drains in ~0.1us at
            # kernel exit (the gpsimd SWDGE path would take ~2.4us).
            nc.sync.dma_start(out=sacc.ap(), in_=strip)

    nc.compile()
    return nc, ucol


_NC = None


def _get_nc():
    global _NC
    if _NC is None:
        _NC = build_nc()
    return _NC


def _l2norm(x):
    return x / np.linalg.norm(x, axis=-1, keepdims=True)


def host_prep(pred1, pred2, target1, target2):
    p1t = _l2norm(np.asarray(pred1, np.float32)).reshape(R, D).T.astype(NPFP8)
    p2t = _l2norm(np.asarray(pred2, np.float32)).reshape(R, D).T.astype(NPFP8)
    t1t = _l2norm(np.asarray(target1, np.float32)).reshape(R, D).T.astype(NPFP8)
    t2t = _l2norm(np.asarray(target2, np.float32)).reshape(R, D).T.astype(NPFP8)
    # Raw own-image diagonal dot blocks (b, n, m), fp8-quantized operands in
    # f32 - the same products the device computes, ~0.4% of total FLOPs.
    pf = [p1t.T.astype(np.float32).reshape(B, N, D),
          p2t.T.astype(np.float32).reshape(B, N, D)]
    tf = [t1t.T.astype(np.float32).reshape(B, N, D),
          t2t.T.astype(np.float32).reshape(B, N, D)]
    diag = [[np.einsum('bnd,bmd->bnm', pf[px], tf[ts]).astype(np.float32)
             for ts in range(2)] for px in range(2)]
    in_maps = []
    for c in range(NCORES):
        r0 = c * RPC
        in_maps.append({
            "pt": np.ascontiguousarray(
                np.concatenate([p1t[:, r0:r0 + RPC], p2t[:, r0:r0 + RPC]],
                               axis=1)),
            "t1t": np.ascontiguousarray(
                np.concatenate([t1t[:, r0:], t1t[:, :r0]], axis=1)),
            "t2t": np.ascontiguousarray(
                np.concatenate([t2t[:, r0:], t2t[:, :r0]], axis=1)),
        })
    return in_maps, diag


def host_post(results, ucol, diag, pind1, pind2, tind1, tind2):
    # S[px, pred]: denominator sums of exp over all 8192 targets.
    S = np.zeros((2, R), np.float64)
    for c, res in enumerate(results):
        r0 = c * RPC
        sa = np.asarray(res["sacc"]).astype(np.float64)   # [128, 32]
        for pt in range(8):
            px, mt = pt // 4, pt % 4
            rows = r0 + mt * 128
            cols = [ucol[(pt, h)] for h in range(4)]
            S[px, rows:rows + 128] += sa[:, cols].sum(axis=1)

    sc = np.float32(SCALE)
    D_aa = sc * diag[0][0]
    D_ab = sc * diag[0][1]
    D_ba = sc * diag[1][0]
    D_bb = sc * diag[1][1]

    f32 = np.float32
    pind1, pind2 = np.asarray(pind1), np.asarray(pind2)
    tind1, tind2 = np.asarray(tind1), np.asarray(tind2)
    same_aa = (pind1[:, :, None] == tind1[:, None, :]).astype(f32)
    same_ab = (pind1[:, :, None] == tind2[:, None, :]).astype(f32)
    same_ba = (pind2[:, :, None] == tind1[:, None, :]).astype(f32)
    same_bb = (pind2[:, :, None] == tind2[:, None, :]).astype(f32)

    S0 = S[0].reshape(B, N)
    S1 = S[1].reshape(B, N)
    # -inf masking correction: both diagonal bands live on ACT units, so
    # the device added exact exps - subtract exact exps.
    corr0 = (same_aa * np.exp(D_aa.astype(np.float64))).sum(-1)
    corr1 = (same_bb * np.exp(D_bb.astype(np.float64))).sum(-1)
    lse0 = np.log(S0 - corr0)
    lse1 = np.log(S1 - corr1)

    num_pos0 = same_ab.sum(-1)
    num_pos1 = same_ba.sum(-1)
    pos_sum0 = (same_ab * D_ab).sum(-1)
    pos_sum1 = (same_ba * D_ba).sum(-1)

    area0 = (pind1[:, :, None] == pind1[:, None, :]).astype(f32).sum(-1)
    area1 = (pind2[:, :, None] == pind2[:, None, :]).astype(f32).sum(-1)
    w0 = (num_pos0 > 0.001).astype(f32) / area0
    w1 = (num_pos1 > 0.001).astype(f32) / area1

    ce0 = -w0 * (pos_sum0 - num_pos0 * lse0) / np.maximum(num_pos0, 1.0)
    ce1 = -w1 * (pos_sum1 - num_pos1 * lse1) / np.maximum(num_pos1, 1.0)
    return np.float32(ce0.mean() + ce1.mean())


def run_hw(inputs, trace=False):
    nc, ucol = _get_nc()
    in_maps, diag = host_prep(inputs["pred1"], inputs["pred2"],
                              inputs["target1"], inputs["target2"])
    last_err = None
    for attempt in range(3):
        try:
            res = run_bass_kernel_spmd(nc, in_maps,
                                       core_ids=list(range(NCORES)),
                                       trace=trace)
            break
        except Exception as e:  # transient NRT device errors recover on retry
            last_err = e
            import time
            time.sleep(20 * (attempt + 1))
    else:
        raise last_err
    loss = host_post(res.results, ucol, diag, inputs["pind1"],
                     inputs["pind2"], inputs["tind1"], inputs["tind2"])
    return loss, res


def kernel(**inputs):
    loss, _ = run_hw(inputs, trace=False)
    return loss


# revision 6
# speedup vs baseline: 1.2651x; 1.1178x over previous
"""DetConB loss (nn_DetConBLoss) on 8 TRN2 NeuronCores via Bass/Tile.

Strategy (data-parallel over batch, targets replicated):
  - Host: l2-normalize preds/targets in f32, flatten to (4096, 256),
    transpose to (d, rows), cast fp8e4m3. Core c owns pred rows
    [c*512, (c+1)*512). Each core receives the full targets with columns
    rolled by c*512 so its own-image diagonal band sits at a fixed,
    compile-time-constant column range (the program is SPMD-identical).
  - Device (per core): 64 units, each a (128 pred x 1024 target) slab:
    fp8 DoubleRow matmuls (K=256 in one pass, f32 PSUM) + one of two
    row-sum consumers, statically balanced to the engines' measured
    rates (ACT ~1.33us/slab incl READ_ACCUMULATOR, DVE ~2.46us/slab):
      * 42 ACT units: exp via ScalarE ACTIVATE with the free in-op
        accumulator (accum_out) - one pass, fused row-sum.
      * 22 DVE units: Schraudolph fast-exp on DVE (int-converting
        multiply-add to an i32 whose bits are the f32 exp) + bitcast
        tensor_reduce. Placed on target slabs that exclude both
        own-image diagonal bands, so the -inf correction on the host
        subtracts exact exps.
    PSUM is split into per-engine ping-pong slot pairs (4 x [128,1024]
    = 8 banks): each engine's successive ops land on alternating slots
    of ITS OWN pair, so a slot refill (2 matmuls, ~0.6us) always hides
    under the other slot's consumer and the engines never hand slots to
    each other (cross-engine handoffs cost ~3us each in the previous
    shared-slot layout).
  - Host: the 16x16 own-image diagonal dot blocks (recomputed from the
    same fp8 inputs, ~0.4% of total FLOPs), masks from the roi indices,
    positive-pair sums, the -inf masking correction (subtract the exp of
    masked entries from the denominators), log, and the final mean.
"""
import numpy as np
import ml_dtypes

import concourse.bacc as bacc
import concourse.mybir as mybir
import concourse.tile as tile
from concourse.bass_utils import run_bass_kernel_spmd

TEMP = 0.1
EPS = 1e-11
SCALE = float(np.float32(1.0 / (TEMP + EPS)))
NCORES = 8
B, N, D = 256, 16, 256
R = B * N          # 4096 flat rows
RPC = R // NCORES  # 512 rows per core
BF16 = mybir.dt.bfloat16
FP8 = mybir.dt.float8e4
NPFP8 = ml_dtypes.float8_e4m3
F32 = mybir.dt.float32
I32 = mybir.dt.int32
# Schraudolph fast-exp: exp(s*x) ~= bitcast_f32(int32(x*SA + SB))
SA = float(np.float32((2**23 / np.log(2.0)) * (1.0 / (0.1 + 1e-11))))
SB = float(np.float32(127 * 2**23 - 486411))

# Unit (pt, q) = pred tile pt (px*4+mt) x target quarter-slab q
# (tsel = q//4, cols [1024*(q%4), +1024)).  DVE units sit on slabs with no
# own-image diagonal (q=0: aa band, q=4: bb band): 42 ACT + 22 DVE.
D_QS = {1: range(8), 5: range(8), 6: range(6)}
D_UNITS = tuple((pt, q) for q, pts in D_QS.items() for pt in pts)
A_UNITS = tuple((pt, q) for q in range(8) for pt in range(8)
                if (pt, q) not in D_UNITS)


def unit_sequence():
    """Interleave ~2 ACT units per DVE unit so both engines are fed from
    the start; each engine's ops alternate over its own PSUM slot pair."""
    a, d, seq = list(A_UNITS), list(D_UNITS), []
    while a or d:
        for _ in range(2):
            if a:
                seq.append((a.pop(0), "A"))
        if d:
            seq.append((d.pop(0), "D"))
    return seq


def build_nc():
    """Build + schedule + compile the SPMD per-core Bass program."""
    nc = bacc.Bacc("TRN2", target_bir_lowering=False, debug=False,
                   num_devices=NCORES)

    p_dram = nc.dram_tensor("pt", [D, 2 * RPC], FP8, kind="ExternalInput")
    t_dram = [nc.dram_tensor(f"t{i + 1}t", [D, R], FP8, kind="ExternalInput")
              for i in range(2)]
    sacc = nc.dram_tensor("sacc", [128, 64], F32, kind="ExternalOutput")

    seq = unit_sequence()
    ucol = {pth: u for u, (pth, _) in enumerate(seq)}

    with tile.TileContext(nc) as tc:
        with (
            tc.tile_pool(name="const", bufs=1) as const_pool,
            tc.tile_pool(name="psum", bufs=1, space="PSUM") as psum_pool,
            tc.tile_pool(name="sch", bufs=2) as sch_pool,
        ):
            # Persistent SBUF: targets as [K=128 partitions, kchunk*R + col],
            # preds as [128, kchunk*1024 + px*512 + col].
            t_sb = [const_pool.tile([128, 2 * R], FP8, name=f"t_sb{i}", tag=f"t{i}")
                    for i in range(2)]
            p_sb = const_pool.tile([128, 2 * 1024], FP8, name="p_sb", tag="p")
            t3 = [t_sb[i].rearrange("p (k c) -> p k c", k=2) for i in range(2)]
            p3 = p_sb.rearrange("p (k c) -> p k c", k=2)

            strip = const_pool.tile([128, 64], F32, name="strip", tag="strip")
            zbias = const_pool.tile([128, 1], F32, name="zbias", tag="zbias")
            scr = const_pool.tile([128, 1024], BF16, name="scr", tag="scr")
            nc.vector.memset(strip, 0.0)
            # Explicit zero-bias AP: a float bias would be lowered through the
            # const-AP machinery, whose TENSOR_LOAD sits in the preamble.
            nc.vector.memset(zbias, 0.0)
            # Warm the exp table set during the input-DMA window so the first
            # real ACTIVATE does not pay the ~2.7us ACT_TABLE_LOAD.
            nc.scalar.activation(strip[:, 0:2], strip[:, 0:2],
                                 mybir.ActivationFunctionType.Exp, bias=zbias)
            nc.vector.memset(strip[:, 0:2], 0.0)

            # PSUM: per-engine ping-pong pairs, 4 x [128, 1024] = 8 banks.
            psA = [psum_pool.tile([128, 1024], F32, name=f"psA{i}", tag=f"psA{i}")
                   for i in range(2)]
            psD = [psum_pool.tile([128, 1024], F32, name=f"psD{i}", tag=f"psD{i}")
                   for i in range(2)]

            # Input DMAs on the sync (HWDGE) queue, ordered by first use in
            # the unit sequence (q-slab first-need order).
            def load_q(q):
                tsel, cs = q // 4, (q % 4) * 1024
                for k in range(2):
                    nc.sync.dma_start(
                        out=t_sb[tsel][:, k * R + cs: k * R + cs + 1024],
                        in_=t_dram[tsel][k * 128:(k + 1) * 128, cs:cs + 1024])

            nc.sync.dma_start(
                out=p3, in_=p_dram.ap().rearrange("(k p) c -> p k c", p=128))
            seen = []
            for (pt, q), _ in seq:
                if q not in seen:
                    seen.append(q)
                    load_q(q)

            na = nd = 0
            for (pt, q), kind in seq:
                tsel, cs = q // 4, (q % 4) * 1024
                i = ucol[(pt, q)]
                ps = psA[na % 2] if kind == "A" else psD[nd % 2]
                lhs = p3[:, :, pt * 128:(pt + 1) * 128]
                # fp8 DoubleRow: both 128-deep K chunks contract in a single
                # pass (lhsT/rhs carry the k pair on a middle AP dim).
                for j in range(2):
                    c0 = cs + j * 512
                    nc.tensor.matmul(
                        ps[:, j * 512:(j + 1) * 512],
                        lhs, t3[tsel][:, :, c0:c0 + 512],
                        start=True, stop=True,
                        perf_mode=mybir.MatmulPerfMode.DoubleRow)
                if kind == "A":
                    nc.scalar.activation(
                        scr, ps, mybir.ActivationFunctionType.Exp,
                        bias=zbias, scale=SCALE,
                        accum_out=strip[:, i:i + 1])
                    na += 1
                else:
                    sch = sch_pool.tile([128, 1024], I32, name="sch", tag="sch")
                    nc.vector.tensor_scalar(
                        sch, ps, SA, SB,
                        op0=mybir.AluOpType.mult, op1=mybir.AluOpType.add)
                    nc.vector.tensor_reduce(
                        strip[:, i:i + 1], sch.bitcast(F32),
                        axis=mybir.AxisListType.X, op=mybir.AluOpType.add)
                    nd += 1
            # Final strip DMA on the sync HWDGE queue: drains in ~0.1us at
            # kernel exit (the gpsimd SWDGE path would take ~2.4us).
            nc.sync.dma_start(out=sacc.ap(), in_=strip)

    nc.compile()
    return nc, ucol


_NC = None


def _get_nc():
    global _NC
    if _NC is None:
        _NC = build_nc()
    return _NC


def _l2norm(x):
    return x / np.linalg.norm(x, axis=-1, keepdims=True)


def host_prep(pred1, pred2, target1, target2):
    p1t = _l2norm(np.asarray(pred1, np.float32)).reshape(R, D).T.astype(NPFP8)
    p2t = _l2norm(np.asarray(pred2, np.float32)).reshape(R, D).T.astype(NPFP8)
    t1t = _l2norm(np.asarray(target1, np.float32)).reshape(R, D).T.astype(NPFP8)
    t2t = _l2norm(np.asarray(target2, np.float32)).reshape(R, D).T.astype(NPFP8)
    # Raw own-image diagonal dot blocks (b, n, m), fp8-quantized operands in
    # f32 - the same products the device computes, ~0.4% of total FLOPs.
    pf = [p1t.T.astype(np.float32).reshape(B, N, D),
          p2t.T.astype(np.float32).reshape(B, N, D)]
    tf = [t1t.T.astype(np.float32).reshape(B, N, D),
          t2t.T.astype(np.float32).reshape(B, N, D)]
    diag = [[np.einsum('bnd,bmd->bnm', pf[px], tf[ts]).astype(np.float32)
             for ts in range(2)] for px in range(2)]
    in_maps = []
    for c in range(NCORES):
        r0 = c * RPC
        in_maps.append({
            "pt": np.ascontiguousarray(
                np.concatenate([p1t[:, r0:r0 + RPC], p2t[:, r0:r0 + RPC]],
                               axis=1)),
            "t1t": np.ascontiguousarray(
                np.concatenate([t1t[:, r0:], t1t[:, :r0]], axis=1)),
            "t2t": np.ascontiguousarray(
                np.concatenate([t2t[:, r0:], t2t[:, :r0]], axis=1)),
        })
    return in_maps, diag


def host_post(results, ucol, diag, pind1, pind2, tind1, tind2):
    # S[px, pred]: denominator sums of exp over all 8192 targets.
    S = np.zeros((2, R), np.float64)
    for c, res in enumerate(results):
        r0 = c * RPC
        sa = np.asarray(res["sacc"]).astype(np.float64)   # [128, 64]
        for pt in range(8):
            px, mt = pt // 4, pt % 4
            rows = r0 + mt * 128
            cols = [ucol[(pt, q)] for q in range(8)]
            S[px, rows:rows + 128] += sa[:, cols].sum(axis=1)

    sc = np.float32(SCALE)
    D_aa = sc * diag[0][0]
    D_ab = sc * diag[0][1]
    D_ba = sc * diag[1][0]
    D_bb = sc * diag[1][1]

    f32 = np.float32
    pind1, pind2 = np.asarray(pind1), np.asarray(pind2)
    tind1, tind2 = np.asarray(tind1), np.asarray(tind2)
    same_aa = (pind1[:, :, None] == tind1[:, None, :]).astype(f32)
    same_ab = (pind1[:, :, None] == tind2[:, None, :]).astype(f32)
    same_ba = (pind2[:, :, None] == tind1[:, None, :]).astype(f32)
    same_bb = (pind2[:, :, None] == tind2[:, None, :]).astype(f32)

    S0 = S[0].reshape(B, N)
    S1 = S[1].reshape(B, N)
    # -inf masking correction: both diagonal bands live on ACT units, so
    # the device added exact exps - subtract exact exps.
    corr0 = (same_aa * np.exp(D_aa.astype(np.float64))).sum(-1)
    corr1 = (same_bb * np.exp(D_bb.astype(np.float64))).sum(-1)
    lse0 = np.log(S0 - corr0)
    lse1 = np.log(S1 - corr1)

    num_pos0 = same_ab.sum(-1)
    num_pos1 = same_ba.sum(-1)
    pos_sum0 = (same_ab * D_ab).sum(-1)
    pos_sum1 = (same_ba * D_ba).sum(-1)

    area0 = (pind1[:, :, None] == pind1[:, None, :]).astype(f32).sum(-1)
    area1 = (pind2[:, :, None] == pind2[:, None, :]).astype(f32).sum(-1)
    w0 = (num_pos0 > 0.001).astype(f32) / area0
    w1 = (num_pos1 > 0.001).astype(f32) / area1

    ce0 = -w0 * (pos_sum0 - num_pos0 * lse0) / np.maximum(num_pos0, 1.0)
    ce1 = -w1 * (pos_sum1 - num_pos1 * lse1) / np.maximum(num_pos1, 1.0)
    return np.float32(ce0.mean() + ce1.mean())


def run_hw(inputs, trace=False):
    nc, ucol = _get_nc()
    in_maps, diag = host_prep(inputs["pred1"], inputs["pred2"],
                              inputs["target1"], inputs["target2"])
    last_err = None
    for attempt in range(3):
        try:
            res = run_bass_kernel_spmd(nc, in_maps,
                                       core_ids=list(range(NCORES)),
                                       trace=trace)
            break
        except Exception as e:  # transient NRT device errors recover on retry
            last_err = e
            import time
            time.sleep(20 * (attempt + 1))
    else:
        raise last_err
    loss = host_post(res.results, ucol, diag, inputs["pind1"],
                     inputs["pind2"], inputs["tind1"], inputs["tind2"])
    return loss, res


def kernel(**inputs):
    loss, _ = run_hw(inputs, trace=False)
    return loss
